# revision 7
# baseline (speedup 1.0000x reference)
"""Multi-head causal self-attention on 8 TRN2 NeuronCores.

Problem: B=2, T=4096, D=512, H=8 heads (hd=64), fp32 in/out.

Sharding: core c in 0..7 handles batch b = c//4 and head pair g = c%4
(heads 2g, 2g+1 -> D-slice [128g, 128g+128)). Each core computes
    partial_out = concat_h( softmax(causal(Q_h K_h^T / 8)) V_h ) @ W_O[slice]
for its two heads; the host sums the 4 partials per batch and adds b_O.

On-core dataflow (all matmul operands bf16, f32 PSUM accumulation):
  - X^T streams in as 8 [128, 4c, 512] per-slice tiles (one DMA each).
  - Q^T,K^T [128(d-pair),4096] = W^T @ X^T, bias added in the PSUM->SBUF
    bf16 copy on DVE. V per 128-t-block in natural layout inside a
    [128, 192] tile laid out [V_A | ones | pad | V_B]: head A's PV
    stationary is cols 0:128 (L_A lands at out-partition 64), head B's is
    cols 64:192 (L_B at 0, Z_B at 64:128) -- both are 128-col aligned
    loads (FWL) and head B needs no partition-shift before the O-proj.
  - Scores are computed transposed, S^T[k-block, q], causally streamed;
    the two heads run concurrently in disjoint 64-row PE groups
    (tile_position). exp() on ScalarE straight out of PSUM with the 1/8
    scale folded in; the diagonal 128x128 subtile is masked AFTER the
    exp by a 0/1 elementwise multiply on DVE (cheaper than identity
    matmuls and keeps the PE row groups conflict-free).
  - Z_aug accumulates P^T-block x V over key blocks in PSUM, one bank per
    head. Scores run one group ahead of the PV matmuls.
  - Normalisation: 1/L on a [128,8] partition-spread layout (DRAM
    bounce), one step-0 broadcast DMA materialises both heads' 1/L rows
    into a [128,512] tile, then znpair = zsb*bc + b_V in two DVE passes.
  - Slices are processed in order 1..7,0 (qkv(0),qkv(1) up front, then
    qkv(s+1), the previous slice's norm chain and O-projection are
    emitted piecewise between score groups so the PE never idles at
    slice boundaries and the epilogue after the final (shortest) slice
    is minimal).
"""

import numpy as np

import concourse.bass as bass
import concourse.mybir as mybir
from concourse.tile import TileContext
from concourse.bass_utils import run_bass_kernel_spmd

try:
    import ml_dtypes

    _BF16 = ml_dtypes.bfloat16
except ImportError:  # pragma: no cover
    _BF16 = None

F32 = mybir.dt.float32
BF16 = mybir.dt.bfloat16

B, T, D, H = 2, 4096, 512, 8
HD = D // H  # 64
SW = 512  # q-slice width
NS = T // SW  # 8 q-slices
NKC = D // 128  # 4 contraction chunks for the projections
NTT = T // 128  # 32 t-tiles / key blocks
GK = 2  # key blocks grouped per exp() call (2 PSUM banks)
SLICE_ORDER = [1, 2, 3, 4, 5, 6, 7, 0]


def _split_waits(nc, max_waits=1):
    """The staged walrus rejects >1 semaphore wait per instruction; hoist
    extras onto same-engine NoOps inserted right before the instruction."""
    counter = 0
    for f in nc.m.functions:
        for blk in f.blocks:
            insts = blk.instructions
            out, changed = [], False
            for ins in insts:
                si = getattr(ins, "sync_info", None)
                waits = list(si.on_wait) if si is not None and si.on_wait else []
                if len(waits) > max_waits:
                    changed = True
                    for w in waits[:-max_waits]:
                        counter += 1
                        nop = mybir.InstNoOp(
                            name=f"I-wsplit-{counter}",
                            engine=ins.engine,
                            ins=[],
                            outs=[],
                        )
                        nop.sync_info = mybir.SyncInfo(on_wait=[w], on_update=[])
                        out.append(nop)
                    ins.sync_info = mybir.SyncInfo(
                        on_wait=waits[-max_waits:], on_update=list(si.on_update)
                    )
                out.append(ins)
            if changed:
                blk.instructions = out
    return counter


def build_nc():
    nc = bass.Bass("TRN2")

    xt = nc.dram_tensor("xt", [D, T], BF16, kind="ExternalInput")
    wqkv = nc.dram_tensor("wqkv", [D, 384], BF16, kind="ExternalInput")
    wo = nc.dram_tensor("wo", [128, D], BF16, kind="ExternalInput")
    bqk = nc.dram_tensor("bqk", [128, 2], F32, kind="ExternalInput")
    bvp = nc.dram_tensor("bvp", [128, 1], F32, kind="ExternalInput")
    out = nc.dram_tensor("out", [T, D], F32, kind="ExternalOutput")

    # mask01[k, q'] = 1 where q' >= k else 0  (S^T diagonal subtile mask)
    mask_np = (
        np.arange(128)[None, :] >= np.arange(128)[:, None]
    ).astype(np.float32)
    mask_dram = nc.inline_tensor(mask_np.astype(_BF16), name="maskc")

    with TileContext(nc) as tc:
        with (
            tc.tile_pool(name="singles", bufs=1) as singles,
            tc.tile_pool(name="sg", bufs=2, space="PSUM") as spool,
            tc.tile_pool(name="pj", bufs=2, space="PSUM") as ppool,
            tc.tile_pool(name="zps", bufs=1, space="PSUM") as zps,
            tc.tile_pool(name="pt", bufs=6) as ptp,
            tc.tile_pool(name="sl", bufs=2) as slp,
            tc.tile_pool(name="outp", bufs=2) as outp,
            tc.tile_pool(name="drp", bufs=2, space="DRAM") as drp,
        ):
            # ---- exp table warm-up (ACT_TABLE_LOAD overlaps the DMAs) ----
            ws = singles.tile([1, 1], F32, tag="ws", name="ws")
            ws2 = singles.tile([1, 1], F32, tag="ws2", name="ws2")
            nc.vector.memset(ws[:, :], 0.0)
            nc.scalar.activation(
                out=ws2[:, :], in_=ws[:, :],
                func=mybir.ActivationFunctionType.Exp,
            )

            # ---- static SBUF + input DMAs (weights first, then X by need) --
            wqkv_sb = singles.tile([128, NKC, 384], BF16, tag="wqkv", name="wqkv_sb")
            nc.sync.dma_start(
                out=wqkv_sb[:, :, :],
                in_=wqkv[:, :].rearrange("(c p) n -> p c n", c=NKC),
            )
            bqk_sb = singles.tile([128, 2], F32, tag="bqk", name="bqk_sb")
            nc.sync.dma_start(out=bqk_sb[:, :], in_=bqk[:, :])

            xt_sb = [None] * NS
            for s in [0, 1]:
                xt_sb[s] = singles.tile([128, NKC, SW], BF16, tag=f"xt{s}", name=f"xt_sb{s}")
                nc.sync.dma_start(
                    out=xt_sb[s][:, :, :],
                    in_=xt[:, s * SW : (s + 1) * SW].rearrange(
                        "(c p) n -> p c n", c=NKC
                    ),
                )
            wo_sb = singles.tile([128, D], BF16, tag="wo", name="wo_sb")
            nc.scalar.dma_start(out=wo_sb[:, :], in_=wo[:, :])
            bvp_sb = singles.tile([128, 1], F32, tag="bvp", name="bvp_sb")
            nc.scalar.dma_start(out=bvp_sb[:, :], in_=bvp[:, :])
            mask_sb = singles.tile([128, 128], BF16, tag="mask", name="mask_sb")
            nc.scalar.dma_start(out=mask_sb[:, :], in_=mask_dram[:, :])
            for s in range(2, NS):
                xt_sb[s] = singles.tile([128, NKC, SW], BF16, tag=f"xt{s}", name=f"xt_sb{s}")
                eng = nc.gpsimd if s % 2 == 0 else nc.scalar
                eng.dma_start(
                    out=xt_sb[s][:, :, :],
                    in_=xt[:, s * SW : (s + 1) * SW].rearrange(
                        "(c p) n -> p c n", c=NKC
                    ),
                )

            qt_sb = [
                singles.tile([128, SW], BF16, tag=f"qt{s}", name=f"qt_sb{s}")
                for s in range(NS)
            ]
            kt_sb = [
                singles.tile([128, SW], BF16, tag=f"kt{s}", name=f"kt_sb{s}")
                for s in range(NS)
            ]
            # V per key block: [128(t), 192] =
            # [V_A(0:64)|ones(64)|pad(65:128)|V_B(128:192)]; pad cols feed
            # only unread Z_aug rows so they stay uninitialised.
            vab_sb = [
                singles.tile([128, 192], BF16, tag=f"vab{t}", name=f"vab_sb{t}")
                for t in range(NTT)
            ]
            for t in range(NTT):
                nc.vector.memset(vab_sb[t][:, 64:65], 1.0)

            # ---- QKV projection pieces (emitted interleaved) ----
            def qkv_pieces(s):
                def emit_q():
                    ps_q = ppool.tile([128, SW], F32, tag="pj", name="ps_q")
                    for c in range(NKC):
                        nc.tensor.matmul(
                            ps_q[:, :],
                            lhsT=wqkv_sb[:, c, 0:128],
                            rhs=xt_sb[s][:, c, :],
                            start=(c == 0),
                            stop=(c == NKC - 1),
                            skip_group_check=True,
                        )
                    nc.vector.tensor_scalar_add(
                        qt_sb[s][:, :], ps_q[:, :], bqk_sb[:, 0:1]
                    )

                def emit_k():
                    ps_k = ppool.tile([128, SW], F32, tag="pj", name="ps_k")
                    for c in range(NKC):
                        nc.tensor.matmul(
                            ps_k[:, :],
                            lhsT=wqkv_sb[:, c, 128:256],
                            rhs=xt_sb[s][:, c, :],
                            start=(c == 0),
                            stop=(c == NKC - 1),
                            skip_group_check=True,
                        )
                    nc.vector.tensor_scalar_add(
                        kt_sb[s][:, :], ps_k[:, :], bqk_sb[:, 1:2]
                    )

                def emit_v(t):
                    def go():
                        tloc = slice((t % 4) * 128, (t % 4 + 1) * 128)
                        ps_v = ppool.tile([128, 128], F32, tag="pj", name="ps_v")
                        for c in range(NKC):
                            nc.tensor.matmul(
                                ps_v[:, :],
                                lhsT=xt_sb[s][:, c, tloc],
                                rhs=wqkv_sb[:, c, 256:384],
                                start=(c == 0),
                                stop=(c == NKC - 1),
                                skip_group_check=True,
                            )
                        nc.vector.tensor_copy(vab_sb[t][:, 0:HD], ps_v[:, 0:HD])
                        nc.vector.tensor_copy(
                            vab_sb[t][:, 128:192], ps_v[:, HD:128]
                        )

                    return go

                return [emit_q, emit_k] + [emit_v(t) for t in range(4 * s, 4 * s + 4)]

            # ---- post-attention pieces for a finished slice ----
            def tail_pieces(s, zaug, zsb, lrow):
                qs = s * SW

                znpair = slp.tile([128, SW], BF16, tag="zn", name="znpair")
                o_big = outp.tile([128, NKC, SW], F32, tag="ot", name="o_big")

                def emit_norm():
                    # L rows -> shared [1,1024]; DRAM bounce to a [128,8]
                    # spread for the reciprocal (DVE iterative divide costs
                    # freedim x 8 cycles); one step-0-partition DMA
                    # broadcasts both heads' 1/L into bc[128, 512].
                    rd = drp.tile([1, 2 * SW], F32, tag="rd", name="rd")
                    nc.gpsimd.dma_start(out=rd[:, :], in_=lrow[:, :])
                    lsp = slp.tile([128, 2 * SW // 128], F32, tag="lsp", name="lsp")
                    nc.gpsimd.dma_start(
                        out=lsp[:, :],
                        in_=rd[0, :].rearrange("(p f) -> p f", p=128),
                    )
                    rsp = slp.tile([128, 2 * SW // 128], F32, tag="rsp", name="rsp")
                    nc.vector.reciprocal(rsp[:, :], lsp[:, :])
                    rd2 = drp.tile([1, 2 * SW], F32, tag="rd2", name="rd2")
                    nc.gpsimd.dma_start(
                        out=rd2[0, :].rearrange("(p f) -> p f", p=128),
                        in_=rsp[:, :],
                    )
                    bc = slp.tile([128, SW], F32, tag="bc", name="bc")
                    rap = rd2[:, :]
                    bcast_src = bass.AP(
                        tensor=rap.tensor,
                        offset=rap.offset,
                        ap=[[SW, 2], [0, HD]] + list(rap.ap[1:-1])
                        + [[1, SW]],
                    )
                    nc.gpsimd.dma_start(out=bc[:, :], in_=bcast_src)
                    nc.vector.tensor_mul(znpair[:, :], zsb[:, :], bc[:, :])
                    nc.vector.tensor_scalar_add(
                        znpair[:, :], znpair[:, :], bvp_sb[:, :]
                    )

                def emit_oproj(j):
                    def go():
                        ps_o = ppool.tile([128, D], F32, tag="pj", name="ps_o")
                        nc.tensor.matmul(
                            ps_o[:, :],
                            lhsT=znpair[:, j * 128 : (j + 1) * 128],
                            rhs=wo_sb[:, :],
                            start=True,
                            stop=True,
                            skip_group_check=True,
                        )
                        nc.vector.tensor_copy(o_big[:, j, :], ps_o[:, :])

                    return go

                def emit_store():
                    nc.sync.dma_start(
                        out=out[qs : qs + SW, :].rearrange(
                            "(j p) n -> p j n", j=NKC
                        ),
                        in_=o_big[:, :, :],
                    )

                # norm runs at end-of-slice; its DMA-bounce latency overlaps
                # >=2 groups of the next slice before the first O-proj matmul
                emit_norm()
                return [], [emit_oproj(j) for j in range(NKC)] + [emit_store]

            # ---- attention ----
            for piece in qkv_pieces(0):
                piece()
            for piece in qkv_pieces(1):
                piece()

            pending = []  # deferred pieces from the previous slice
            for idx, s in enumerate(SLICE_ORDER):
                qs = s * SW
                nkb = 4 * (s + 1)
                zaug = [
                    zps.tile([128, SW], F32, tag="za", name="zauga"),
                    zps.tile([128, SW], F32, tag="zb", name="zaugb"),
                ]
                # pack key blocks tightly into groups; a matmul output may
                # not cross a PSUM bank boundary, so bump to the next bank
                # when a block would straddle one
                groups, cur, cur_cols = [], [], 0
                for kb in range(nkb):
                    qlo = max(qs, kb * 128)
                    n = qs + SW - qlo
                    off = cur_cols
                    if off % SW + n > SW:
                        off = ((off + SW - 1) // SW) * SW
                    if off + n > GK * SW:
                        groups.append(cur)
                        cur, off = [], 0
                    cur.append((kb, off, n, qlo))
                    cur_cols = off + n
                if cur:
                    groups.append(cur)

                # piece schedule: norm(prev) at group 0, qkv(next) spread
                # over early groups, oproj(prev)+store(prev) over late groups
                # (no earlier than group 2, after the norm bounce has landed)
                front = list(pending[0]) if pending else []
                back = list(pending[1]) if pending else []
                pending = []
                if idx < 6:
                    front.extend(qkv_pieces(SLICE_ORDER[idx + 1]))
                ng = len(groups)
                sched = [[] for _ in range(ng)]
                for i, p in enumerate(front):
                    g = min(i * max(ng - 1, 1) // max(len(front), 1), ng - 1)
                    sched[g].append(p)
                g0 = min(2, ng - 1)
                for i, p in enumerate(back):
                    g = min(g0 + i * max(ng - g0, 1) // max(len(back), 1), ng - 1)
                    sched[g].append(p)

                def emit_av(av):
                    pt_t, grp_t = av
                    for h in range(2):
                        vcols = (slice(0, 128), slice(64, 192))[h]
                        for kb, off, n, qlo in grp_t:
                            nc.tensor.matmul(
                                zaug[h][:, qlo - qs : SW],
                                lhsT=vab_sb[kb][:, vcols],
                                rhs=pt_t[h][:, off : off + n],
                                start=(kb == 0),
                                stop=(kb == nkb - 1),
                                skip_group_check=True,
                            )

                av_queue = []
                for gi, grp in enumerate(groups):
                    used = grp[-1][1] + grp[-1][2]
                    sg = [None, None]
                    pt = [None, None]
                    for h in range(2):
                        sg[h] = spool.tile([128, GK * SW], F32, tag="sg", name="sg")
                        pt[h] = ptp.tile([128, GK * SW], BF16, tag="pt", name="pt")
                    # scores (both heads -> disjoint PE row groups, run
                    # concurrently; no full-row matmuls in between)
                    hrows = (slice(0, HD), slice(HD, 128))
                    for kb, off, n, qlo in grp:
                        for h in range(2):
                            nc.tensor.matmul(
                                sg[h][:, off : off + n],
                                lhsT=kt_sb[kb // 4][
                                    hrows[h], (kb % 4) * 128 : (kb % 4 + 1) * 128
                                ],
                                rhs=qt_sb[s][hrows[h], qlo - qs : qlo - qs + n],
                                start=True,
                                stop=True,
                                skip_group_check=True,
                                tile_position=(h * HD, 0),
                            )
                    for h in range(2):
                        nc.scalar.activation(
                            out=pt[h][:, 0:used],
                            in_=sg[h][:, 0:used],
                            func=mybir.ActivationFunctionType.Exp,
                            scale=0.125,
                        )
                    # diagonal subtile causal mask: zero q < k after the exp
                    for kb, off, n, qlo in grp:
                        if kb * 128 >= qs:
                            for h in range(2):
                                nc.vector.tensor_mul(
                                    pt[h][:, off : off + 128],
                                    pt[h][:, off : off + 128],
                                    mask_sb[:, :],
                                )
                    for p in sched[gi]:
                        p()
                    av_queue.append((pt, grp))
                    if len(av_queue) > 1:
                        emit_av(av_queue.pop(0))
                while av_queue:
                    emit_av(av_queue.pop(0))

                # evacuate Z and the L rows promptly (frees the PSUM banks
                # for the next slice); L_A sits at row 64 of zaug[0], L_B at
                # row 63 of zaug[1], Z_B already at partitions 64..127.
                zsb = slp.tile([128, SW], F32, tag="zsb", name="zsb")
                nc.vector.tensor_copy(zsb[0:HD, :], zaug[0][0:HD, :])
                nc.vector.tensor_copy(zsb[HD:128, :], zaug[1][HD:128, :])
                lrow = slp.tile([1, 2 * SW], F32, tag="lr", name="lrow")
                nc.vector.tensor_copy(lrow[0:1, 0:SW], zaug[0][HD : HD + 1, :])
                nc.vector.tensor_copy(lrow[0:1, SW : 2 * SW], zaug[1][0:1, :])

                pending = tail_pieces(s, zaug, zsb, lrow)

            for piece in pending[0] + pending[1]:
                piece()

    _split_waits(nc)
    return nc


_NC_CACHE = {}


def _get_nc():
    if "nc" not in _NC_CACHE:
        _NC_CACHE["nc"] = build_nc()
    return _NC_CACHE["nc"]


def make_in_maps(combined_embed, W_K, b_K, W_Q, b_Q, W_V, b_V, W_O, b_O):
    f32 = np.float32
    in_maps = []
    for c in range(8):
        b = c // 4
        g = c % 4
        sl = slice(g * 128, (g + 1) * 128)
        xt = np.ascontiguousarray(np.asarray(combined_embed[b], f32).T)
        wqkv = np.hstack(
            [
                np.asarray(W_Q, f32)[:, sl],
                np.asarray(W_K, f32)[:, sl],
                np.asarray(W_V, f32)[:, sl],
            ]
        )
        bqk = np.stack([np.asarray(b_Q, f32)[sl], np.asarray(b_K, f32)[sl]], 1)
        in_maps.append(
            {
                "xt": xt.astype(_BF16),
                "wqkv": np.ascontiguousarray(wqkv).astype(_BF16),
                "wo": np.ascontiguousarray(np.asarray(W_O, f32)[sl, :]).astype(
                    _BF16
                ),
                "bqk": np.ascontiguousarray(bqk),
                "bvp": np.asarray(b_V, f32)[sl].reshape(128, 1).copy(),
            }
        )
    return in_maps


def run_cores(in_maps, **kwargs):
    nc = _get_nc()
    return run_bass_kernel_spmd(nc, in_maps, core_ids=list(range(8)), **kwargs)


def kernel(
    combined_embed, W_K, b_K, W_Q, b_Q, W_V, b_V, W_O, b_O
):  # full inputs -> full output
    in_maps = make_in_maps(
        combined_embed, W_K, b_K, W_Q, b_Q, W_V, b_V, W_O, b_O
    )
    res = run_cores(in_maps)
    out = np.zeros((B, T, D), np.float32)
    for c in range(8):
        out[c // 4] += res.results[c]["out"]
    out += np.asarray(b_O, np.float32)[None, None, :]
    return out


# revision 8
# speedup vs baseline: 1.0594x; 1.0594x over previous
"""Multi-head causal self-attention on 8 TRN2 NeuronCores.

Problem: B=2, T=4096, D=512, H=8 heads (hd=64), fp32 in/out.

Sharding: core c in 0..7 handles batch b = c//4 and head pair g = c%4
(heads 2g, 2g+1 -> D-slice [128g, 128g+128)). Each core computes
    partial_out = concat_h( softmax(causal(Q_h K_h^T / 8)) V_h ) @ W_O[slice]
for its two heads; the host sums the 4 partials per batch and adds b_O.

On-core dataflow (all matmul operands bf16, f32 PSUM accumulation):
  - X^T streams in as 8 [128, 4c, 512] per-slice tiles (one DMA each).
  - Q^T,K^T [128(d-pair),4096] = W^T @ X^T, bias added in the PSUM->SBUF
    bf16 copy on DVE. V per 128-t-block in natural layout inside a
    [128, 192] tile laid out [V_A | ones | pad | V_B]: head A's PV
    stationary is cols 0:128 (L_A lands at out-partition 64), head B's is
    cols 64:192 (L_B at 0, Z_B at 64:128) -- both are 128-col aligned
    loads (FWL) and head B needs no partition-shift before the O-proj.
  - Scores are computed transposed, S^T[k-block, q], causally streamed;
    the two heads run concurrently in disjoint 64-row PE groups
    (tile_position). exp() on ScalarE straight out of PSUM with the 1/8
    scale folded in; the diagonal 128x128 subtile is masked AFTER the
    exp by a 0/1 elementwise multiply on DVE (cheaper than identity
    matmuls and keeps the PE row groups conflict-free).
  - Z_aug accumulates P^T-block x V over key blocks in PSUM, one bank per
    head. Scores run one group ahead of the PV matmuls.
  - Normalisation: 1/L on a [128,8] partition-spread layout (DRAM
    bounce), one step-0 broadcast DMA materialises both heads' 1/L rows
    into a [128,512] tile, then znpair = zsb*bc + b_V in two DVE passes.
  - Slices are processed in order 1..7,0 (qkv(0),qkv(1) up front, then
    qkv(s+1), the previous slice's norm chain and O-projection are
    emitted piecewise between score groups so the PE never idles at
    slice boundaries and the epilogue after the final (shortest) slice
    is minimal).
"""

import numpy as np

import concourse.bass as bass
import concourse.mybir as mybir
from concourse.tile import TileContext
from concourse.bass_utils import run_bass_kernel_spmd

try:
    import ml_dtypes

    _BF16 = ml_dtypes.bfloat16
except ImportError:  # pragma: no cover
    _BF16 = None

F32 = mybir.dt.float32
BF16 = mybir.dt.bfloat16

B, T, D, H = 2, 4096, 512, 8
HD = D // H  # 64
SW = 512  # q-slice width
NS = T // SW  # 8 q-slices
NKC = D // 128  # 4 contraction chunks for the projections
NTT = T // 128  # 32 t-tiles / key blocks
GK = 2  # key blocks grouped per exp() call (2 PSUM banks)
SLICE_ORDER = [1, 2, 3, 4, 5, 6, 7, 0]


def _split_waits(nc, max_waits=1):
    """The staged walrus rejects >1 semaphore wait per instruction; hoist
    extras onto same-engine NoOps inserted right before the instruction."""
    counter = 0
    for f in nc.m.functions:
        for blk in f.blocks:
            insts = blk.instructions
            out, changed = [], False
            for ins in insts:
                si = getattr(ins, "sync_info", None)
                waits = list(si.on_wait) if si is not None and si.on_wait else []
                if len(waits) > max_waits:
                    changed = True
                    for w in waits[:-max_waits]:
                        counter += 1
                        nop = mybir.InstNoOp(
                            name=f"I-wsplit-{counter}",
                            engine=ins.engine,
                            ins=[],
                            outs=[],
                        )
                        nop.sync_info = mybir.SyncInfo(on_wait=[w], on_update=[])
                        out.append(nop)
                    ins.sync_info = mybir.SyncInfo(
                        on_wait=waits[-max_waits:], on_update=list(si.on_update)
                    )
                out.append(ins)
            if changed:
                blk.instructions = out
    return counter


def build_nc():
    nc = bass.Bass("TRN2")

    xt = nc.dram_tensor("xt", [D, T], BF16, kind="ExternalInput")
    wqkv = nc.dram_tensor("wqkv", [D, 384], BF16, kind="ExternalInput")
    wo = nc.dram_tensor("wo", [128, D], BF16, kind="ExternalInput")
    bqk = nc.dram_tensor("bqk", [128, 2], F32, kind="ExternalInput")
    bvp = nc.dram_tensor("bvp", [128, 1], F32, kind="ExternalInput")
    out = nc.dram_tensor("out", [T, D], F32, kind="ExternalOutput")

    # mask01[k, q'] = 1 where q' >= k else 0  (S^T diagonal subtile mask)
    mask_np = (
        np.arange(128)[None, :] >= np.arange(128)[:, None]
    ).astype(np.float32)
    mask_dram = nc.inline_tensor(mask_np.astype(_BF16), name="maskc")

    with TileContext(nc) as tc:
        with (
            tc.tile_pool(name="singles", bufs=1) as singles,
            tc.tile_pool(name="sg", bufs=2, space="PSUM") as spool,
            tc.tile_pool(name="pj", bufs=2, space="PSUM") as ppool,
            tc.tile_pool(name="zps", bufs=1, space="PSUM") as zps,
            tc.tile_pool(name="pt", bufs=6) as ptp,
            tc.tile_pool(name="sl", bufs=2) as slp,
            tc.tile_pool(name="outp", bufs=2) as outp,
            tc.tile_pool(name="drp", bufs=2, space="DRAM") as drp,
        ):
            # ---- exp table warm-up (ACT_TABLE_LOAD overlaps the DMAs) ----
            ws = singles.tile([1, 1], F32, tag="ws", name="ws")
            ws2 = singles.tile([1, 1], F32, tag="ws2", name="ws2")
            nc.vector.memset(ws[:, :], 0.0)
            nc.scalar.activation(
                out=ws2[:, :], in_=ws[:, :],
                func=mybir.ActivationFunctionType.Exp,
            )

            # ---- static SBUF + input DMAs (weights first, then X by need) --
            wqkv_sb = singles.tile([128, NKC, 384], BF16, tag="wqkv", name="wqkv_sb")
            nc.sync.dma_start(
                out=wqkv_sb[:, :, :],
                in_=wqkv[:, :].rearrange("(c p) n -> p c n", c=NKC),
            )
            bqk_sb = singles.tile([128, 2], F32, tag="bqk", name="bqk_sb")
            nc.sync.dma_start(out=bqk_sb[:, :], in_=bqk[:, :])

            xt_sb = [None] * NS
            for s in [0, 1]:
                xt_sb[s] = singles.tile([128, NKC, SW], BF16, tag=f"xt{s}", name=f"xt_sb{s}")
                nc.sync.dma_start(
                    out=xt_sb[s][:, :, :],
                    in_=xt[:, s * SW : (s + 1) * SW].rearrange(
                        "(c p) n -> p c n", c=NKC
                    ),
                )
            wo_sb = singles.tile([128, D], BF16, tag="wo", name="wo_sb")
            nc.scalar.dma_start(out=wo_sb[:, :], in_=wo[:, :])
            bvp_sb = singles.tile([128, 1], F32, tag="bvp", name="bvp_sb")
            nc.scalar.dma_start(out=bvp_sb[:, :], in_=bvp[:, :])
            mask_sb = singles.tile([128, 128], BF16, tag="mask", name="mask_sb")
            nc.scalar.dma_start(out=mask_sb[:, :], in_=mask_dram[:, :])
            for s in range(2, NS):
                xt_sb[s] = singles.tile([128, NKC, SW], BF16, tag=f"xt{s}", name=f"xt_sb{s}")
                eng = nc.gpsimd if s % 2 == 0 else nc.scalar
                eng.dma_start(
                    out=xt_sb[s][:, :, :],
                    in_=xt[:, s * SW : (s + 1) * SW].rearrange(
                        "(c p) n -> p c n", c=NKC
                    ),
                )

            qt_sb = [
                singles.tile([128, SW], BF16, tag=f"qt{s}", name=f"qt_sb{s}")
                for s in range(NS)
            ]
            kt_sb = [
                singles.tile([128, SW], BF16, tag=f"kt{s}", name=f"kt_sb{s}")
                for s in range(NS)
            ]
            # V per key block: [128(t), 192] =
            # [V_A(0:64)|ones(64)|pad(65:128)|V_B(128:192)]; pad cols feed
            # only unread Z_aug rows so they stay uninitialised.
            vab_sb = [
                singles.tile([128, 192], BF16, tag=f"vab{t}", name=f"vab_sb{t}")
                for t in range(NTT)
            ]
            for t in range(NTT):
                nc.vector.memset(vab_sb[t][:, 64:65], 1.0)

            # ---- QKV projection pieces (emitted interleaved) ----
            def qkv_pieces(s):
                def emit_q():
                    ps_q = ppool.tile([128, SW], F32, tag="pj", name="ps_q")
                    for c in range(NKC):
                        nc.tensor.matmul(
                            ps_q[:, :],
                            lhsT=wqkv_sb[:, c, 0:128],
                            rhs=xt_sb[s][:, c, :],
                            start=(c == 0),
                            stop=(c == NKC - 1),
                            skip_group_check=True,
                        )
                    nc.vector.tensor_scalar_add(
                        qt_sb[s][:, :], ps_q[:, :], bqk_sb[:, 0:1]
                    )

                def emit_k():
                    ps_k = ppool.tile([128, SW], F32, tag="pj", name="ps_k")
                    for c in range(NKC):
                        nc.tensor.matmul(
                            ps_k[:, :],
                            lhsT=wqkv_sb[:, c, 128:256],
                            rhs=xt_sb[s][:, c, :],
                            start=(c == 0),
                            stop=(c == NKC - 1),
                            skip_group_check=True,
                        )
                    nc.vector.tensor_scalar_add(
                        kt_sb[s][:, :], ps_k[:, :], bqk_sb[:, 1:2]
                    )

                def emit_v(t):
                    def go():
                        tloc = slice((t % 4) * 128, (t % 4 + 1) * 128)
                        ps_v = ppool.tile([128, 128], F32, tag="pj", name="ps_v")
                        for c in range(NKC):
                            nc.tensor.matmul(
                                ps_v[:, :],
                                lhsT=xt_sb[s][:, c, tloc],
                                rhs=wqkv_sb[:, c, 256:384],
                                start=(c == 0),
                                stop=(c == NKC - 1),
                                skip_group_check=True,
                            )
                        nc.vector.tensor_copy(vab_sb[t][:, 0:HD], ps_v[:, 0:HD])
                        nc.vector.tensor_copy(
                            vab_sb[t][:, 128:192], ps_v[:, HD:128]
                        )

                    return go

                return [emit_q, emit_k] + [emit_v(t) for t in range(4 * s, 4 * s + 4)]

            # ---- post-attention pieces for a finished slice ----
            def tail_pieces(s, zaug, zsb, lrow):
                qs = s * SW

                znpair = slp.tile([128, SW], BF16, tag="zn", name="znpair")
                o_big = outp.tile([128, NKC, SW], F32, tag="ot", name="o_big")

                def emit_norm():
                    # L rows -> shared [1,1024]; DRAM bounce to a [128,8]
                    # spread for the reciprocal (DVE iterative divide costs
                    # freedim x 8 cycles); one step-0-partition DMA
                    # broadcasts both heads' 1/L into bc[128, 512].
                    rd = drp.tile([1, 2 * SW], F32, tag="rd", name="rd")
                    nc.gpsimd.dma_start(out=rd[:, :], in_=lrow[:, :])
                    lsp = slp.tile([128, 2 * SW // 128], F32, tag="lsp", name="lsp")
                    nc.gpsimd.dma_start(
                        out=lsp[:, :],
                        in_=rd[0, :].rearrange("(p f) -> p f", p=128),
                    )
                    rsp = slp.tile([128, 2 * SW // 128], F32, tag="rsp", name="rsp")
                    nc.vector.reciprocal(rsp[:, :], lsp[:, :])
                    rd2 = drp.tile([1, 2 * SW], F32, tag="rd2", name="rd2")
                    nc.gpsimd.dma_start(
                        out=rd2[0, :].rearrange("(p f) -> p f", p=128),
                        in_=rsp[:, :],
                    )
                    bc = slp.tile([128, SW], F32, tag="bc", name="bc")
                    rap = rd2[:, :]
                    bcast_src = bass.AP(
                        tensor=rap.tensor,
                        offset=rap.offset,
                        ap=[[SW, 2], [0, HD]] + list(rap.ap[1:-1])
                        + [[1, SW]],
                    )
                    nc.gpsimd.dma_start(out=bc[:, :], in_=bcast_src)

                    def mul_piece():
                        nc.vector.tensor_mul(znpair[:, :], zsb[:, :], bc[:, :])
                        nc.vector.tensor_scalar_add(
                            znpair[:, :], znpair[:, :], bvp_sb[:, :]
                        )

                    return mul_piece

                def emit_oproj(j):
                    def go():
                        ps_o = ppool.tile([128, D], F32, tag="pj", name="ps_o")
                        nc.tensor.matmul(
                            ps_o[:, :],
                            lhsT=znpair[:, j * 128 : (j + 1) * 128],
                            rhs=wo_sb[:, :],
                            start=True,
                            stop=True,
                            skip_group_check=True,
                        )
                        nc.vector.tensor_copy(o_big[:, j, :], ps_o[:, :])

                    return go

                def emit_store():
                    nc.sync.dma_start(
                        out=out[qs : qs + SW, :].rearrange(
                            "(j p) n -> p j n", j=NKC
                        ),
                        in_=o_big[:, :, :],
                    )

                # the norm DMA chain launches at end-of-slice; the DVE
                # multiply becomes a piece (group >=1 of the next slice) so the
                # in-order DVE queue never stalls on the bounce latency, and
                # the O-proj matmuls go later still (group >=2)
                mul_piece = emit_norm()
                return [mul_piece], [emit_oproj(j) for j in range(NKC)] + [
                    emit_store
                ]

            # ---- attention ----
            for piece in qkv_pieces(0):
                piece()
            for piece in qkv_pieces(1):
                piece()

            pending = []  # deferred pieces from the previous slice
            for idx, s in enumerate(SLICE_ORDER):
                qs = s * SW
                nkb = 4 * (s + 1)
                zaug = [
                    zps.tile([128, SW], F32, tag="za", name="zauga"),
                    zps.tile([128, SW], F32, tag="zb", name="zaugb"),
                ]
                # pack key blocks tightly into groups; a matmul output may
                # not cross a PSUM bank boundary, so bump to the next bank
                # when a block would straddle one
                groups, cur, cur_cols = [], [], 0
                for kb in range(nkb):
                    qlo = max(qs, kb * 128)
                    n = qs + SW - qlo
                    off = cur_cols
                    if off % SW + n > SW:
                        off = ((off + SW - 1) // SW) * SW
                    if off + n > GK * SW:
                        groups.append(cur)
                        cur, off = [], 0
                    cur.append((kb, off, n, qlo))
                    cur_cols = off + n
                if cur:
                    groups.append(cur)

                # piece schedule: norm(prev) at group 0, qkv(next) spread
                # over early groups, oproj(prev)+store(prev) over late groups
                # (no earlier than group 2, after the norm bounce has landed)
                front = list(pending[0]) if pending else []
                back = list(pending[1]) if pending else []
                pending = []
                if idx < 6:
                    front.extend(qkv_pieces(SLICE_ORDER[idx + 1]))
                ng = len(groups)
                sched = [[] for _ in range(ng)]
                for i, p in enumerate(front):
                    g = min(1 + i * max(ng - 2, 1) // max(len(front), 1), ng - 1)
                    sched[g].append(p)
                g0 = min(2, ng - 1)
                for i, p in enumerate(back):
                    g = min(g0 + i * max(ng - g0, 1) // max(len(back), 1), ng - 1)
                    sched[g].append(p)

                def emit_av(av):
                    pt_t, grp_t = av
                    for h in range(2):
                        vcols = (slice(0, 128), slice(64, 192))[h]
                        for kb, off, n, qlo in grp_t:
                            nc.tensor.matmul(
                                zaug[h][:, qlo - qs : SW],
                                lhsT=vab_sb[kb][:, vcols],
                                rhs=pt_t[h][:, off : off + n],
                                start=(kb == 0),
                                stop=(kb == nkb - 1),
                                skip_group_check=True,
                            )

                av_queue = []
                for gi, grp in enumerate(groups):
                    used = grp[-1][1] + grp[-1][2]
                    sg = [None, None]
                    pt = [None, None]
                    for h in range(2):
                        sg[h] = spool.tile([128, GK * SW], F32, tag="sg", name="sg")
                        pt[h] = ptp.tile([128, GK * SW], BF16, tag="pt", name="pt")
                    # scores (both heads -> disjoint PE row groups, run
                    # concurrently; no full-row matmuls in between)
                    hrows = (slice(0, HD), slice(HD, 128))
                    for kb, off, n, qlo in grp:
                        for h in range(2):
                            nc.tensor.matmul(
                                sg[h][:, off : off + n],
                                lhsT=kt_sb[kb // 4][
                                    hrows[h], (kb % 4) * 128 : (kb % 4 + 1) * 128
                                ],
                                rhs=qt_sb[s][hrows[h], qlo - qs : qlo - qs + n],
                                start=True,
                                stop=True,
                                skip_group_check=True,
                                tile_position=(h * HD, 0),
                            )
                    for h in range(2):
                        nc.scalar.activation(
                            out=pt[h][:, 0:used],
                            in_=sg[h][:, 0:used],
                            func=mybir.ActivationFunctionType.Exp,
                            scale=0.125,
                        )
                    # diagonal subtile causal mask: zero q < k after the exp
                    for kb, off, n, qlo in grp:
                        if kb * 128 >= qs:
                            for h in range(2):
                                nc.vector.tensor_mul(
                                    pt[h][:, off : off + 128],
                                    pt[h][:, off : off + 128],
                                    mask_sb[:, :],
                                )
                    for p in sched[gi]:
                        p()
                    av_queue.append((pt, grp))
                    if len(av_queue) > 1:
                        emit_av(av_queue.pop(0))
                while av_queue:
                    emit_av(av_queue.pop(0))

                # evacuate Z and the L rows promptly (frees the PSUM banks
                # for the next slice); L_A sits at row 64 of zaug[0], L_B at
                # row 63 of zaug[1], Z_B already at partitions 64..127.
                zsb = slp.tile([128, SW], F32, tag="zsb", name="zsb")
                nc.vector.tensor_copy(zsb[0:HD, :], zaug[0][0:HD, :])
                nc.vector.tensor_copy(zsb[HD:128, :], zaug[1][HD:128, :])
                lrow = slp.tile([1, 2 * SW], F32, tag="lr", name="lrow")
                nc.vector.tensor_copy(lrow[0:1, 0:SW], zaug[0][HD : HD + 1, :])
                nc.vector.tensor_copy(lrow[0:1, SW : 2 * SW], zaug[1][0:1, :])

                pending = tail_pieces(s, zaug, zsb, lrow)

            for piece in pending[0] + pending[1]:
                piece()

    _split_waits(nc)
    return nc


_NC_CACHE = {}


def _get_nc():
    if "nc" not in _NC_CACHE:
        _NC_CACHE["nc"] = build_nc()
    return _NC_CACHE["nc"]


def make_in_maps(combined_embed, W_K, b_K, W_Q, b_Q, W_V, b_V, W_O, b_O):
    f32 = np.float32
    in_maps = []
    for c in range(8):
        b = c // 4
        g = c % 4
        sl = slice(g * 128, (g + 1) * 128)
        xt = np.ascontiguousarray(np.asarray(combined_embed[b], f32).T)
        wqkv = np.hstack(
            [
                np.asarray(W_Q, f32)[:, sl],
                np.asarray(W_K, f32)[:, sl],
                np.asarray(W_V, f32)[:, sl],
            ]
        )
        bqk = np.stack([np.asarray(b_Q, f32)[sl], np.asarray(b_K, f32)[sl]], 1)
        in_maps.append(
            {
                "xt": xt.astype(_BF16),
                "wqkv": np.ascontiguousarray(wqkv).astype(_BF16),
                "wo": np.ascontiguousarray(np.asarray(W_O, f32)[sl, :]).astype(
                    _BF16
                ),
                "bqk": np.ascontiguousarray(bqk),
                "bvp": np.asarray(b_V, f32)[sl].reshape(128, 1).copy(),
            }
        )
    return in_maps


def run_cores(in_maps, **kwargs):
    nc = _get_nc()
    return run_bass_kernel_spmd(nc, in_maps, core_ids=list(range(8)), **kwargs)


def kernel(
    combined_embed, W_K, b_K, W_Q, b_Q, W_V, b_V, W_O, b_O
):  # full inputs -> full output
    in_maps = make_in_maps(
        combined_embed, W_K, b_K, W_Q, b_Q, W_V, b_V, W_O, b_O
    )
    res = run_cores(in_maps)
    out = np.zeros((B, T, D), np.float32)
    for c in range(8):
        out[c // 4] += res.results[c]["out"]
    out += np.asarray(b_O, np.float32)[None, None, :]
    return out


# revision 9
# speedup vs baseline: 1.0746x; 1.0143x over previous
"""Multi-head causal self-attention on 8 TRN2 NeuronCores.

Problem: B=2, T=4096, D=512, H=8 heads (hd=64), fp32 in/out.

Sharding: core c in 0..7 handles batch b = c//4 and head pair g = c%4
(heads 2g, 2g+1 -> D-slice [128g, 128g+128)). Each core computes
    partial_out = concat_h( softmax(causal(Q_h K_h^T / 8)) V_h ) @ W_O[slice]
for its two heads; the host sums the 4 partials per batch and adds b_O.

On-core dataflow (all matmul operands bf16, f32 PSUM accumulation):
  - X^T streams in as 8 [128, 4c, 512] per-slice tiles (one DMA each).
  - Q^T,K^T [128(d-pair),4096] = W^T @ X^T, bias added in the PSUM->SBUF
    bf16 copy on DVE. V per 128-t-block in natural layout inside a
    [128, 192] tile laid out [V_A | ones | pad | V_B]: head A's PV
    stationary is cols 0:128 (L_A lands at out-partition 64), head B's is
    cols 64:192 (L_B at 0, Z_B at 64:128) -- both are 128-col aligned
    loads (FWL) and head B needs no partition-shift before the O-proj.
  - Scores are computed transposed, S^T[k-block, q], causally streamed;
    the two heads run concurrently in disjoint 64-row PE groups
    (tile_position). exp() on ScalarE straight out of PSUM with the 1/8
    scale folded in; the diagonal 128x128 subtile is masked AFTER the
    exp by a 0/1 elementwise multiply on DVE (cheaper than identity
    matmuls and keeps the PE row groups conflict-free).
  - Z_aug accumulates P^T-block x V over key blocks in PSUM, one bank per
    head. Scores run one group ahead of the PV matmuls.
  - Normalisation: 1/L on a [128,8] partition-spread layout (DRAM
    bounce), one step-0 broadcast DMA materialises both heads' 1/L rows
    into a [128,512] tile, then znpair = zsb*bc + b_V in two DVE passes.
  - Slices are processed in order 1..7,0 (qkv(0),qkv(1) up front, then
    qkv(s+1), the previous slice's norm chain and O-projection are
    emitted piecewise between score groups so the PE never idles at
    slice boundaries and the epilogue after the final (shortest) slice
    is minimal).
"""

import numpy as np

import concourse.bass as bass
import concourse.mybir as mybir
from concourse.tile import TileContext
from concourse.bass_utils import run_bass_kernel_spmd

try:
    import ml_dtypes

    _BF16 = ml_dtypes.bfloat16
except ImportError:  # pragma: no cover
    _BF16 = None

F32 = mybir.dt.float32
BF16 = mybir.dt.bfloat16

B, T, D, H = 2, 4096, 512, 8
HD = D // H  # 64
SW = 512  # q-slice width
NS = T // SW  # 8 q-slices
NKC = D // 128  # 4 contraction chunks for the projections
NTT = T // 128  # 32 t-tiles / key blocks
GK = 2  # key blocks grouped per exp() call (2 PSUM banks)
SLICE_ORDER = [1, 2, 3, 4, 5, 6, 7, 0]


def _split_waits(nc, max_waits=1):
    """The staged walrus rejects >1 semaphore wait per instruction; hoist
    extras onto same-engine NoOps inserted right before the instruction."""
    counter = 0
    for f in nc.m.functions:
        for blk in f.blocks:
            insts = blk.instructions
            out, changed = [], False
            for ins in insts:
                si = getattr(ins, "sync_info", None)
                waits = list(si.on_wait) if si is not None and si.on_wait else []
                if len(waits) > max_waits:
                    changed = True
                    for w in waits[:-max_waits]:
                        counter += 1
                        nop = mybir.InstNoOp(
                            name=f"I-wsplit-{counter}",
                            engine=ins.engine,
                            ins=[],
                            outs=[],
                        )
                        nop.sync_info = mybir.SyncInfo(on_wait=[w], on_update=[])
                        out.append(nop)
                    ins.sync_info = mybir.SyncInfo(
                        on_wait=waits[-max_waits:], on_update=list(si.on_update)
                    )
                out.append(ins)
            if changed:
                blk.instructions = out
    return counter


def build_nc():
    nc = bass.Bass("TRN2")

    xt = nc.dram_tensor("xt", [D, T], BF16, kind="ExternalInput")
    wqkv = nc.dram_tensor("wqkv", [D, 384], BF16, kind="ExternalInput")
    wo = nc.dram_tensor("wo", [128, D], BF16, kind="ExternalInput")
    bqk = nc.dram_tensor("bqk", [128, 2], F32, kind="ExternalInput")
    bvp = nc.dram_tensor("bvp", [128, 1], F32, kind="ExternalInput")
    out = nc.dram_tensor("out", [T, D], F32, kind="ExternalOutput")

    # mask01[k, q'] = 1 where q' >= k else 0  (S^T diagonal subtile mask)
    mask_np = (
        np.arange(128)[None, :] >= np.arange(128)[:, None]
    ).astype(np.float32)
    mask_dram = nc.inline_tensor(mask_np.astype(_BF16), name="maskc")

    with TileContext(nc) as tc:
        with (
            tc.tile_pool(name="singles", bufs=1) as singles,
            tc.tile_pool(name="sg", bufs=2, space="PSUM") as spool,
            tc.tile_pool(name="pj", bufs=2, space="PSUM") as ppool,
            tc.tile_pool(name="zps", bufs=1, space="PSUM") as zps,
            tc.tile_pool(name="pt", bufs=6) as ptp,
            tc.tile_pool(name="sl", bufs=3) as slp,
            tc.tile_pool(name="outp", bufs=2) as outp,
            tc.tile_pool(name="drp", bufs=3, space="DRAM") as drp,
        ):
            # ---- exp table warm-up (ACT_TABLE_LOAD overlaps the DMAs) ----
            ws = singles.tile([1, 1], F32, tag="ws", name="ws")
            ws2 = singles.tile([1, 1], F32, tag="ws2", name="ws2")
            nc.vector.memset(ws[:, :], 0.0)
            nc.scalar.activation(
                out=ws2[:, :], in_=ws[:, :],
                func=mybir.ActivationFunctionType.Exp,
            )

            # ---- static SBUF + input DMAs (weights first, then X by need) --
            wqkv_sb = singles.tile([128, NKC, 384], BF16, tag="wqkv", name="wqkv_sb")
            nc.sync.dma_start(
                out=wqkv_sb[:, :, :],
                in_=wqkv[:, :].rearrange("(c p) n -> p c n", c=NKC),
            )
            bqk_sb = singles.tile([128, 2], F32, tag="bqk", name="bqk_sb")
            nc.sync.dma_start(out=bqk_sb[:, :], in_=bqk[:, :])

            xt_sb = [None] * NS
            for s in [0, 1]:
                xt_sb[s] = singles.tile([128, NKC, SW], BF16, tag=f"xt{s}", name=f"xt_sb{s}")
                nc.sync.dma_start(
                    out=xt_sb[s][:, :, :],
                    in_=xt[:, s * SW : (s + 1) * SW].rearrange(
                        "(c p) n -> p c n", c=NKC
                    ),
                )
            wo_sb = singles.tile([128, D], BF16, tag="wo", name="wo_sb")
            nc.scalar.dma_start(out=wo_sb[:, :], in_=wo[:, :])
            bvp_sb = singles.tile([128, 1], F32, tag="bvp", name="bvp_sb")
            nc.scalar.dma_start(out=bvp_sb[:, :], in_=bvp[:, :])
            mask_sb = singles.tile([128, 128], BF16, tag="mask", name="mask_sb")
            nc.scalar.dma_start(out=mask_sb[:, :], in_=mask_dram[:, :])
            for s in range(2, NS):
                xt_sb[s] = singles.tile([128, NKC, SW], BF16, tag=f"xt{s}", name=f"xt_sb{s}")
                eng = nc.gpsimd if s % 2 == 0 else nc.scalar
                eng.dma_start(
                    out=xt_sb[s][:, :, :],
                    in_=xt[:, s * SW : (s + 1) * SW].rearrange(
                        "(c p) n -> p c n", c=NKC
                    ),
                )

            qt_sb = [
                singles.tile([128, SW], BF16, tag=f"qt{s}", name=f"qt_sb{s}")
                for s in range(NS)
            ]
            kt_sb = [
                singles.tile([128, SW], BF16, tag=f"kt{s}", name=f"kt_sb{s}")
                for s in range(NS)
            ]
            # V per key block: [128(t), 192] =
            # [V_A(0:64)|ones(64)|pad(65:128)|V_B(128:192)]; pad cols feed
            # only unread Z_aug rows so they stay uninitialised.
            vab_sb = [
                singles.tile([128, 192], BF16, tag=f"vab{t}", name=f"vab_sb{t}")
                for t in range(NTT)
            ]
            for t in range(NTT):
                nc.vector.memset(vab_sb[t][:, 64:65], 1.0)

            # ---- QKV projection pieces (emitted interleaved) ----
            def qkv_pieces(s):
                def emit_q():
                    ps_q = ppool.tile([128, SW], F32, tag="pj", name="ps_q")
                    for c in range(NKC):
                        nc.tensor.matmul(
                            ps_q[:, :],
                            lhsT=wqkv_sb[:, c, 0:128],
                            rhs=xt_sb[s][:, c, :],
                            start=(c == 0),
                            stop=(c == NKC - 1),
                            skip_group_check=True,
                        )
                    nc.vector.tensor_scalar_add(
                        qt_sb[s][:, :], ps_q[:, :], bqk_sb[:, 0:1]
                    )

                def emit_k():
                    ps_k = ppool.tile([128, SW], F32, tag="pj", name="ps_k")
                    for c in range(NKC):
                        nc.tensor.matmul(
                            ps_k[:, :],
                            lhsT=wqkv_sb[:, c, 128:256],
                            rhs=xt_sb[s][:, c, :],
                            start=(c == 0),
                            stop=(c == NKC - 1),
                            skip_group_check=True,
                        )
                    nc.vector.tensor_scalar_add(
                        kt_sb[s][:, :], ps_k[:, :], bqk_sb[:, 1:2]
                    )

                def emit_v(t):
                    def go():
                        tloc = slice((t % 4) * 128, (t % 4 + 1) * 128)
                        ps_v = ppool.tile([128, 128], F32, tag="pj", name="ps_v")
                        for c in range(NKC):
                            nc.tensor.matmul(
                                ps_v[:, :],
                                lhsT=xt_sb[s][:, c, tloc],
                                rhs=wqkv_sb[:, c, 256:384],
                                start=(c == 0),
                                stop=(c == NKC - 1),
                                skip_group_check=True,
                            )
                        nc.vector.tensor_copy(vab_sb[t][:, 0:HD], ps_v[:, 0:HD])
                        nc.vector.tensor_copy(
                            vab_sb[t][:, 128:192], ps_v[:, HD:128]
                        )

                    return go

                return [emit_q, emit_k] + [emit_v(t) for t in range(4 * s, 4 * s + 4)]

            # ---- post-attention pieces for a finished slice ----
            def tail_pieces(s, zaug, zsb, lrow):
                qs = s * SW

                znpair = slp.tile([128, SW], BF16, tag="zn", name="znpair")
                o_big = outp.tile([128, NKC, SW], F32, tag="ot", name="o_big")

                def emit_norm():
                    # L rows -> shared [1,1024]; DRAM bounce to a [128,8]
                    # spread for the reciprocal (DVE iterative divide costs
                    # freedim x 8 cycles); one step-0-partition DMA
                    # broadcasts both heads' 1/L into bc[128, 512].
                    rd = drp.tile([1, 2 * SW], F32, tag="rd", name="rd")
                    nc.sync.dma_start(out=rd[:, :], in_=lrow[:, :])
                    lsp = slp.tile([128, 2 * SW // 128], F32, tag="lsp", name="lsp")
                    nc.sync.dma_start(
                        out=lsp[:, :],
                        in_=rd[0, :].rearrange("(p f) -> p f", p=128),
                    )
                    rsp = slp.tile([128, 2 * SW // 128], F32, tag="rsp", name="rsp")
                    nc.vector.reciprocal(rsp[:, :], lsp[:, :])
                    rd2 = drp.tile([1, 2 * SW], F32, tag="rd2", name="rd2")
                    nc.sync.dma_start(
                        out=rd2[0, :].rearrange("(p f) -> p f", p=128),
                        in_=rsp[:, :],
                    )
                    bc = slp.tile([128, SW], F32, tag="bc", name="bc")
                    rap = rd2[:, :]
                    bcast_src = bass.AP(
                        tensor=rap.tensor,
                        offset=rap.offset,
                        ap=[[SW, 2], [0, HD]] + list(rap.ap[1:-1])
                        + [[1, SW]],
                    )
                    nc.sync.dma_start(out=bc[:, :], in_=bcast_src)

                    def mul_piece():
                        nc.vector.tensor_mul(znpair[:, :], zsb[:, :], bc[:, :])
                        nc.vector.tensor_scalar_add(
                            znpair[:, :], znpair[:, :], bvp_sb[:, :]
                        )

                    return mul_piece

                def emit_oproj(j):
                    def go():
                        ps_o = ppool.tile([128, D], F32, tag="pj", name="ps_o")
                        nc.tensor.matmul(
                            ps_o[:, :],
                            lhsT=znpair[:, j * 128 : (j + 1) * 128],
                            rhs=wo_sb[:, :],
                            start=True,
                            stop=True,
                            skip_group_check=True,
                        )
                        nc.vector.tensor_copy(o_big[:, j, :], ps_o[:, :])

                    return go

                def emit_store():
                    nc.sync.dma_start(
                        out=out[qs : qs + SW, :].rearrange(
                            "(j p) n -> p j n", j=NKC
                        ),
                        in_=o_big[:, :, :],
                    )

                # the norm DMA chain launches at end-of-slice; the DVE
                # multiply and the O-proj run TWO slices later, giving the
                # bounce a full slice of runway so no engine queue-head ever
                # waits on it (list-scheduler inversions included)
                mul_piece = emit_norm()
                return [], [mul_piece] + [emit_oproj(j) for j in range(NKC)] + [
                    emit_store
                ]

            # ---- attention ----
            for piece in qkv_pieces(0):
                piece()
            for piece in qkv_pieces(1):
                piece()

            pending = []  # (front, back) piece lists, consumed 2 slices later
            for idx, s in enumerate(SLICE_ORDER):
                qs = s * SW
                nkb = 4 * (s + 1)
                zaug = [
                    zps.tile([128, SW], F32, tag="za", name="zauga"),
                    zps.tile([128, SW], F32, tag="zb", name="zaugb"),
                ]
                # pack key blocks tightly into groups; a matmul output may
                # not cross a PSUM bank boundary, so bump to the next bank
                # when a block would straddle one
                groups, cur, cur_cols = [], [], 0
                for kb in range(nkb):
                    qlo = max(qs, kb * 128)
                    n = qs + SW - qlo
                    off = cur_cols
                    if off % SW + n > SW:
                        off = ((off + SW - 1) // SW) * SW
                    if off + n > GK * SW:
                        groups.append(cur)
                        cur, off = [], 0
                    cur.append((kb, off, n, qlo))
                    cur_cols = off + n
                if cur:
                    groups.append(cur)

                # piece schedule: norm(prev) at group 0, qkv(next) spread
                # over early groups, oproj(prev)+store(prev) over late groups
                # (no earlier than group 2, after the norm bounce has landed)
                front = []
                back = []
                if len(pending) == 2:  # back-pieces from two slices ago
                    back = list(pending.pop(0)[1])
                if idx < 6:
                    front.extend(qkv_pieces(SLICE_ORDER[idx + 1]))
                ng = len(groups)
                sched = [[] for _ in range(ng)]
                for i, p in enumerate(front):
                    g = min(1 + i * max(ng - 2, 1) // max(len(front), 1), ng - 1)
                    sched[g].append(p)
                for i, p in enumerate(back):
                    g = min(
                        (0 if i == 0 else 2 + (i - 1) * max(ng - 2, 1) // 5),
                        ng - 1,
                    )
                    sched[g].append(p)

                def emit_av(av):
                    pt_t, grp_t = av
                    for h in range(2):
                        vcols = (slice(0, 128), slice(64, 192))[h]
                        for kb, off, n, qlo in grp_t:
                            nc.tensor.matmul(
                                zaug[h][:, qlo - qs : SW],
                                lhsT=vab_sb[kb][:, vcols],
                                rhs=pt_t[h][:, off : off + n],
                                start=(kb == 0),
                                stop=(kb == nkb - 1),
                                skip_group_check=True,
                            )

                av_queue = []
                for gi, grp in enumerate(groups):
                    used = grp[-1][1] + grp[-1][2]
                    sg = [None, None]
                    pt = [None, None]
                    for h in range(2):
                        sg[h] = spool.tile([128, GK * SW], F32, tag="sg", name="sg")
                        pt[h] = ptp.tile([128, GK * SW], BF16, tag="pt", name="pt")
                    # scores (both heads -> disjoint PE row groups, run
                    # concurrently; no full-row matmuls in between)
                    hrows = (slice(0, HD), slice(HD, 128))
                    for kb, off, n, qlo in grp:
                        for h in range(2):
                            nc.tensor.matmul(
                                sg[h][:, off : off + n],
                                lhsT=kt_sb[kb // 4][
                                    hrows[h], (kb % 4) * 128 : (kb % 4 + 1) * 128
                                ],
                                rhs=qt_sb[s][hrows[h], qlo - qs : qlo - qs + n],
                                start=True,
                                stop=True,
                                skip_group_check=True,
                                tile_position=(h * HD, 0),
                            )
                    for h in range(2):
                        nc.scalar.activation(
                            out=pt[h][:, 0:used],
                            in_=sg[h][:, 0:used],
                            func=mybir.ActivationFunctionType.Exp,
                            scale=0.125,
                        )
                    # diagonal subtile causal mask: zero q < k after the exp
                    for kb, off, n, qlo in grp:
                        if kb * 128 >= qs:
                            for h in range(2):
                                nc.vector.tensor_mul(
                                    pt[h][:, off : off + 128],
                                    pt[h][:, off : off + 128],
                                    mask_sb[:, :],
                                )
                    for p in sched[gi]:
                        p()
                    av_queue.append((pt, grp))
                    if len(av_queue) > 1:
                        emit_av(av_queue.pop(0))
                while av_queue:
                    emit_av(av_queue.pop(0))

                # evacuate Z and the L rows promptly (frees the PSUM banks
                # for the next slice); L_A sits at row 64 of zaug[0], L_B at
                # row 63 of zaug[1], Z_B already at partitions 64..127.
                zsb = slp.tile([128, SW], F32, tag="zsb", name="zsb")
                nc.vector.tensor_copy(zsb[0:HD, :], zaug[0][0:HD, :])
                nc.vector.tensor_copy(zsb[HD:128, :], zaug[1][HD:128, :])
                lrow = slp.tile([1, 2 * SW], F32, tag="lr", name="lrow")
                nc.vector.tensor_copy(lrow[0:1, 0:SW], zaug[0][HD : HD + 1, :])
                nc.vector.tensor_copy(lrow[0:1, SW : 2 * SW], zaug[1][0:1, :])

                pending.append(tail_pieces(s, zaug, zsb, lrow))

            for fr, bk in pending:
                for piece in fr + bk:
                    piece()

    _split_waits(nc)
    return nc


_NC_CACHE = {}


def _get_nc():
    if "nc" not in _NC_CACHE:
        _NC_CACHE["nc"] = build_nc()
    return _NC_CACHE["nc"]


def make_in_maps(combined_embed, W_K, b_K, W_Q, b_Q, W_V, b_V, W_O, b_O):
    f32 = np.float32
    in_maps = []
    for c in range(8):
        b = c // 4
        g = c % 4
        sl = slice(g * 128, (g + 1) * 128)
        xt = np.ascontiguousarray(np.asarray(combined_embed[b], f32).T)
        wqkv = np.hstack(
            [
                np.asarray(W_Q, f32)[:, sl],
                np.asarray(W_K, f32)[:, sl],
                np.asarray(W_V, f32)[:, sl],
            ]
        )
        bqk = np.stack([np.asarray(b_Q, f32)[sl], np.asarray(b_K, f32)[sl]], 1)
        in_maps.append(
            {
                "xt": xt.astype(_BF16),
                "wqkv": np.ascontiguousarray(wqkv).astype(_BF16),
                "wo": np.ascontiguousarray(np.asarray(W_O, f32)[sl, :]).astype(
                    _BF16
                ),
                "bqk": np.ascontiguousarray(bqk),
                "bvp": np.asarray(b_V, f32)[sl].reshape(128, 1).copy(),
            }
        )
    return in_maps


def run_cores(in_maps, **kwargs):
    nc = _get_nc()
    return run_bass_kernel_spmd(nc, in_maps, core_ids=list(range(8)), **kwargs)


def kernel(
    combined_embed, W_K, b_K, W_Q, b_Q, W_V, b_V, W_O, b_O
):  # full inputs -> full output
    in_maps = make_in_maps(
        combined_embed, W_K, b_K, W_Q, b_Q, W_V, b_V, W_O, b_O
    )
    res = run_cores(in_maps)
    out = np.zeros((B, T, D), np.float32)
    for c in range(8):
        out[c // 4] += res.results[c]["out"]
    out += np.asarray(b_O, np.float32)[None, None, :]
    return out


# revision 10
# speedup vs baseline: 1.2244x; 1.1394x over previous
"""Multi-head causal self-attention on 8 TRN2 NeuronCores.

Problem: B=2, T=4096, D=512, H=8 heads (hd=64), fp32 in/out.

Sharding: core c in 0..7 handles batch b = c//4 and head pair g = c%4
(heads 2g, 2g+1 -> D-slice [128g, 128g+128)). Each core computes
    partial_out = concat_h( softmax(causal(Q_h K_h^T / 8)) V_h ) @ W_O[slice]
for its two heads; the host sums the 4 partials per batch and adds b_O.

On-core dataflow (all matmul operands bf16, f32 PSUM accumulation):
  - X^T streams in as 8 [128, 4c, 512] per-slice tiles (one DMA each).
  - Q^T,K^T [128(d-pair),4096] = W^T @ X^T, bias added in the PSUM->SBUF
    bf16 copy on DVE. V per 128-t-block in natural layout inside a
    [128, 192] tile laid out [V_A | ones | pad | V_B]: head A's PV
    stationary is cols 0:128 (L_A lands at out-partition 64), head B's is
    cols 64:192 (L_B at 0, Z_B at 64:128) -- both are 128-col aligned
    loads (FWL) and head B needs no partition-shift before the O-proj.
  - Scores are computed transposed, S^T[k-block, q], causally streamed;
    the two heads run concurrently in disjoint 64-row PE groups
    (tile_position). exp() on ScalarE straight out of PSUM with the 1/8
    scale folded in; the diagonal 128x128 subtile is masked AFTER the
    exp by a 0/1 elementwise multiply on DVE (cheaper than identity
    matmuls and keeps the PE row groups conflict-free).
  - Z_aug accumulates P^T-block x V over key blocks in PSUM, one bank per
    head. Scores run one group ahead of the PV matmuls.
  - Normalisation: 1/L on a [128,8] partition-spread layout (DRAM
    bounce), one step-0 broadcast DMA materialises both heads' 1/L rows
    into a [128,512] tile, then znpair = zsb*bc + b_V in two DVE passes.
  - Slices are processed in order 1..7,0 (qkv(0),qkv(1) up front, then
    qkv(s+1), the previous slice's norm chain and O-projection are
    emitted piecewise between score groups so the PE never idles at
    slice boundaries and the epilogue after the final (shortest) slice
    is minimal).
"""

import numpy as np

import concourse.bass as bass
import concourse.mybir as mybir
from concourse.tile import TileContext
from concourse.bass_utils import run_bass_kernel_spmd

try:
    import ml_dtypes

    _BF16 = ml_dtypes.bfloat16
except ImportError:  # pragma: no cover
    _BF16 = None

F32 = mybir.dt.float32
BF16 = mybir.dt.bfloat16

B, T, D, H = 2, 4096, 512, 8
HD = D // H  # 64
SW = 512  # q-slice width
NS = T // SW  # 8 q-slices
NKC = D // 128  # 4 contraction chunks for the projections
NTT = T // 128  # 32 t-tiles / key blocks
GK = 2  # key blocks grouped per exp() call (2 PSUM banks)
SLICE_ORDER = [1, 2, 3, 4, 5, 6, 7, 0]


def _split_waits(nc, max_waits=1):
    """The staged walrus rejects >1 semaphore wait per instruction; hoist
    extras onto same-engine NoOps inserted right before the instruction."""
    counter = 0
    for f in nc.m.functions:
        for blk in f.blocks:
            insts = blk.instructions
            out, changed = [], False
            for ins in insts:
                si = getattr(ins, "sync_info", None)
                waits = list(si.on_wait) if si is not None and si.on_wait else []
                if len(waits) > max_waits:
                    changed = True
                    for w in waits[:-max_waits]:
                        counter += 1
                        nop = mybir.InstNoOp(
                            name=f"I-wsplit-{counter}",
                            engine=ins.engine,
                            ins=[],
                            outs=[],
                        )
                        nop.sync_info = mybir.SyncInfo(on_wait=[w], on_update=[])
                        out.append(nop)
                    ins.sync_info = mybir.SyncInfo(
                        on_wait=waits[-max_waits:], on_update=list(si.on_update)
                    )
                out.append(ins)
            if changed:
                blk.instructions = out
    return counter


def build_nc():
    nc = bass.Bass("TRN2")

    xt = nc.dram_tensor("xt", [D, T], BF16, kind="ExternalInput")
    wqkv = nc.dram_tensor("wqkv", [D, 384], BF16, kind="ExternalInput")
    wo = nc.dram_tensor("wo", [128, D], BF16, kind="ExternalInput")
    bqk = nc.dram_tensor("bqk", [128, 2], F32, kind="ExternalInput")
    bvp = nc.dram_tensor("bvp", [128, 1], F32, kind="ExternalInput")
    out = nc.dram_tensor("out", [T, D], F32, kind="ExternalOutput")

    # mask01[k, q'] = 1 where q' >= k else 0  (S^T diagonal subtile mask)
    mask_np = (
        np.arange(128)[None, :] >= np.arange(128)[:, None]
    ).astype(np.float32)
    mask_dram = nc.inline_tensor(mask_np.astype(_BF16), name="maskc")

    with TileContext(nc) as tc:
        with (
            tc.tile_pool(name="singles", bufs=1) as singles,
            tc.tile_pool(name="sg", bufs=2, space="PSUM") as spool,
            tc.tile_pool(name="pj", bufs=2, space="PSUM") as ppool,
            tc.tile_pool(name="zps", bufs=1, space="PSUM") as zps,
            tc.tile_pool(name="pt", bufs=6) as ptp,
            tc.tile_pool(name="sl", bufs=3) as slp,
            tc.tile_pool(name="outp", bufs=2) as outp,
            tc.tile_pool(name="drp", bufs=3, space="DRAM") as drp,
        ):
            # ---- exp table warm-up (ACT_TABLE_LOAD overlaps the DMAs) ----
            ws = singles.tile([1, 1], F32, tag="ws", name="ws")
            ws2 = singles.tile([1, 1], F32, tag="ws2", name="ws2")
            nc.vector.memset(ws[:, :], 0.0)
            nc.scalar.activation(
                out=ws2[:, :], in_=ws[:, :],
                func=mybir.ActivationFunctionType.Exp,
            )

            # ---- static SBUF + input DMAs (weights first, then X by need) --
            wqkv_sb = singles.tile([128, NKC, 384], BF16, tag="wqkv", name="wqkv_sb")
            nc.sync.dma_start(
                out=wqkv_sb[:, :, :],
                in_=wqkv[:, :].rearrange("(c p) n -> p c n", c=NKC),
            )
            bqk_sb = singles.tile([128, 2], F32, tag="bqk", name="bqk_sb")
            nc.sync.dma_start(out=bqk_sb[:, :], in_=bqk[:, :])

            xt_sb = [None] * NS
            for s in [0, 1]:
                xt_sb[s] = singles.tile([128, NKC, SW], BF16, tag=f"xt{s}", name=f"xt_sb{s}")
                nc.sync.dma_start(
                    out=xt_sb[s][:, :, :],
                    in_=xt[:, s * SW : (s + 1) * SW].rearrange(
                        "(c p) n -> p c n", c=NKC
                    ),
                )
            wo_sb = singles.tile([128, D], BF16, tag="wo", name="wo_sb")
            nc.scalar.dma_start(out=wo_sb[:, :], in_=wo[:, :])
            bvp_sb = singles.tile([128, 1], F32, tag="bvp", name="bvp_sb")
            nc.scalar.dma_start(out=bvp_sb[:, :], in_=bvp[:, :])
            mask_sb = singles.tile([128, 128], BF16, tag="mask", name="mask_sb")
            nc.scalar.dma_start(out=mask_sb[:, :], in_=mask_dram[:, :])
            for s in range(2, NS):
                xt_sb[s] = singles.tile([128, NKC, SW], BF16, tag=f"xt{s}", name=f"xt_sb{s}")
                eng = nc.gpsimd if s % 2 == 0 else nc.scalar
                eng.dma_start(
                    out=xt_sb[s][:, :, :],
                    in_=xt[:, s * SW : (s + 1) * SW].rearrange(
                        "(c p) n -> p c n", c=NKC
                    ),
                )

            qt_sb = [
                singles.tile([128, SW], BF16, tag=f"qt{s}", name=f"qt_sb{s}")
                for s in range(NS)
            ]
            kt_sb = [
                singles.tile([128, SW], BF16, tag=f"kt{s}", name=f"kt_sb{s}")
                for s in range(NS)
            ]
            # V per key block: [128(t), 192] =
            # [V_A(0:64)|ones(64)|pad(65:128)|V_B(128:192)]; pad cols feed
            # only unread Z_aug rows so they stay uninitialised.
            vab_sb = [
                singles.tile([128, 192], BF16, tag=f"vab{t}", name=f"vab_sb{t}")
                for t in range(NTT)
            ]
            for t in range(NTT):
                nc.vector.memset(vab_sb[t][:, 64:65], 1.0)

            # ---- QKV projection pieces (emitted interleaved) ----
            def qkv_pieces(s):
                def emit_q():
                    ps_q = ppool.tile([128, SW], F32, tag="pj", name="ps_q")
                    for c in range(NKC):
                        nc.tensor.matmul(
                            ps_q[:, :],
                            lhsT=wqkv_sb[:, c, 0:128],
                            rhs=xt_sb[s][:, c, :],
                            start=(c == 0),
                            stop=(c == NKC - 1),
                            skip_group_check=True,
                        )
                    nc.vector.tensor_scalar_add(
                        qt_sb[s][:, :], ps_q[:, :], bqk_sb[:, 0:1]
                    )

                def emit_k():
                    ps_k = ppool.tile([128, SW], F32, tag="pj", name="ps_k")
                    for c in range(NKC):
                        nc.tensor.matmul(
                            ps_k[:, :],
                            lhsT=wqkv_sb[:, c, 128:256],
                            rhs=xt_sb[s][:, c, :],
                            start=(c == 0),
                            stop=(c == NKC - 1),
                            skip_group_check=True,
                        )
                    nc.vector.tensor_scalar_add(
                        kt_sb[s][:, :], ps_k[:, :], bqk_sb[:, 1:2]
                    )

                def emit_v(t):
                    def go():
                        tloc = slice((t % 4) * 128, (t % 4 + 1) * 128)
                        ps_v = ppool.tile([128, 128], F32, tag="pj", name="ps_v")
                        for c in range(NKC):
                            nc.tensor.matmul(
                                ps_v[:, :],
                                lhsT=xt_sb[s][:, c, tloc],
                                rhs=wqkv_sb[:, c, 256:384],
                                start=(c == 0),
                                stop=(c == NKC - 1),
                                skip_group_check=True,
                            )
                        nc.vector.tensor_copy(vab_sb[t][:, 0:HD], ps_v[:, 0:HD])
                        nc.vector.tensor_copy(
                            vab_sb[t][:, 128:192], ps_v[:, HD:128]
                        )

                    return go

                return [emit_q, emit_k] + [emit_v(t) for t in range(4 * s, 4 * s + 4)]

            # ---- post-attention pieces for a finished slice ----
            def tail_pieces(s, zaug, zsb, lrow):
                qs = s * SW

                znpair = slp.tile([128, SW], BF16, tag="zn", name="znpair")
                o_big = outp.tile([128, NKC, SW], F32, tag="ot", name="o_big")

                def emit_norm():
                    # L rows -> shared [1,1024]; DRAM bounce to a [128,8]
                    # spread for the reciprocal (DVE iterative divide costs
                    # freedim x 8 cycles); one step-0-partition DMA
                    # broadcasts both heads' 1/L into bc[128, 512].
                    rd = drp.tile([1, 2 * SW], F32, tag="rd", name="rd")
                    nc.sync.dma_start(out=rd[:, :], in_=lrow[:, :])
                    lsp = slp.tile([128, 2 * SW // 128], F32, tag="lsp", name="lsp")
                    nc.sync.dma_start(
                        out=lsp[:, :],
                        in_=rd[0, :].rearrange("(p f) -> p f", p=128),
                    )
                    rsp = slp.tile([128, 2 * SW // 128], F32, tag="rsp", name="rsp")
                    nc.vector.reciprocal(rsp[:, :], lsp[:, :])
                    rd2 = drp.tile([1, 2 * SW], F32, tag="rd2", name="rd2")
                    nc.sync.dma_start(
                        out=rd2[0, :].rearrange("(p f) -> p f", p=128),
                        in_=rsp[:, :],
                    )
                    bc = slp.tile([128, SW], F32, tag="bc", name="bc")
                    rap = rd2[:, :]
                    bcast_src = bass.AP(
                        tensor=rap.tensor,
                        offset=rap.offset,
                        ap=[[SW, 2], [0, HD]] + list(rap.ap[1:-1])
                        + [[1, SW]],
                    )
                    nc.sync.dma_start(out=bc[:, :], in_=bcast_src)

                    def mul_piece():
                        nc.vector.tensor_mul(znpair[:, :], zsb[:, :], bc[:, :])
                        nc.vector.tensor_scalar_add(
                            znpair[:, :], znpair[:, :], bvp_sb[:, :]
                        )

                    return mul_piece

                def emit_oproj(j):
                    def go():
                        ps_o = ppool.tile([128, D], F32, tag="pj", name="ps_o")
                        nc.tensor.matmul(
                            ps_o[:, :],
                            lhsT=znpair[:, j * 128 : (j + 1) * 128],
                            rhs=wo_sb[:, :],
                            start=True,
                            stop=True,
                            skip_group_check=True,
                        )
                        nc.vector.tensor_copy(o_big[:, j, :], ps_o[:, :])

                    return go

                def emit_store():
                    nc.sync.dma_start(
                        out=out[qs : qs + SW, :].rearrange(
                            "(j p) n -> p j n", j=NKC
                        ),
                        in_=o_big[:, :, :],
                    )

                # the norm DMA chain launches at end-of-slice; the DVE
                # multiply and the O-proj run TWO slices later, giving the
                # bounce a full slice of runway so no engine queue-head ever
                # waits on it (list-scheduler inversions included)
                mul_piece = emit_norm()
                return [], [mul_piece] + [emit_oproj(j) for j in range(NKC)] + [
                    emit_store
                ]

            # ---- attention ----
            for piece in qkv_pieces(0):
                piece()
            for piece in qkv_pieces(1):
                piece()

            pending = []  # (front, back) piece lists, consumed 2 slices later
            hrows = (slice(0, HD), slice(HD, 128))
            for idx, s in enumerate(SLICE_ORDER):
                qs = s * SW
                nkb = 4 * (s + 1)
                zaug = [
                    zps.tile([128, SW], F32, tag="za", name="zauga"),
                    zps.tile([128, SW], F32, tag="zb", name="zaugb"),
                ]
                # piece schedule: qkv(next) spread over blocks, norm-mul of
                # two slices ago at block 0, its O-proj from block ~4 on
                front = []
                back = []
                if len(pending) == 2:  # back-pieces from two slices ago
                    back = list(pending.pop(0)[1])
                if idx < 6:
                    front.extend(qkv_pieces(SLICE_ORDER[idx + 1]))
                sched = [[] for _ in range(nkb)]
                for i, p in enumerate(front):
                    g = min(1 + i * max(nkb - 2, 1) // max(len(front), 1), nkb - 1)
                    sched[g].append(p)
                for i, p in enumerate(back):
                    g = min(
                        (0 if i == 0 else 3 + (i - 1) * max(nkb - 3, 1) // 5),
                        nkb - 1,
                    )
                    sched[g].append(p)

                def emit_av(av):
                    pt_t, kb, n, qlo = av
                    for h in range(2):
                        vcols = (slice(0, 128), slice(64, 192))[h]
                        nc.tensor.matmul(
                            zaug[h][:, qlo - qs : SW],
                            lhsT=vab_sb[kb][:, vcols],
                            rhs=pt_t[:, h, 0:n],
                            start=(kb == 0),
                            stop=(kb == nkb - 1),
                            skip_group_check=True,
                        )

                av_queue = []
                for kb in range(nkb):
                    qlo = max(qs, kb * 128)
                    n = qs + SW - qlo
                    # both heads' scores share one [128, 2, 512] PSUM tile
                    # (one bank per head): a single allocation per block, so
                    # the pair issues back-to-back with no semaphore between
                    # the two matmuls (disjoint PE row groups -> concurrent)
                    sg = spool.tile([128, 2, SW], F32, tag="sg", name="sg")
                    pt = ptp.tile([128, 2, SW], BF16, tag="pt", name="pt")
                    for h in range(2):
                        nc.tensor.matmul(
                            sg[:, h, 0:n],
                            lhsT=kt_sb[kb // 4][
                                hrows[h], (kb % 4) * 128 : (kb % 4 + 1) * 128
                            ],
                            rhs=qt_sb[s][hrows[h], qlo - qs : qlo - qs + n],
                            start=True,
                            stop=True,
                            skip_group_check=True,
                            tile_position=(h * HD, 0),
                        )
                    # one exp covers both heads (3-D access pattern)
                    nc.scalar.activation(
                        out=pt[:, :, 0:n],
                        in_=sg[:, :, 0:n],
                        func=mybir.ActivationFunctionType.Exp,
                        scale=0.125,
                    )
                    # diagonal subtile causal mask: zero q < k after the exp
                    if kb * 128 >= qs:
                        for h in range(2):
                            nc.vector.tensor_mul(
                                pt[:, h, 0:128],
                                pt[:, h, 0:128],
                                mask_sb[:, :],
                            )
                    for p in sched[kb]:
                        p()
                    av_queue.append((pt, kb, n, qlo))
                    if len(av_queue) > 1:
                        emit_av(av_queue.pop(0))
                while av_queue:
                    emit_av(av_queue.pop(0))

                # evacuate Z and the L rows promptly (frees the PSUM banks
                # for the next slice); L_A sits at row 64 of zaug[0], L_B at
                # row 63 of zaug[1], Z_B already at partitions 64..127.
                zsb = slp.tile([128, SW], F32, tag="zsb", name="zsb")
                nc.vector.tensor_copy(zsb[0:HD, :], zaug[0][0:HD, :])
                nc.vector.tensor_copy(zsb[HD:128, :], zaug[1][HD:128, :])
                lrow = slp.tile([1, 2 * SW], F32, tag="lr", name="lrow")
                nc.vector.tensor_copy(lrow[0:1, 0:SW], zaug[0][HD : HD + 1, :])
                nc.vector.tensor_copy(lrow[0:1, SW : 2 * SW], zaug[1][0:1, :])

                pending.append(tail_pieces(s, zaug, zsb, lrow))

            for fr, bk in pending:
                for piece in fr + bk:
                    piece()

    _split_waits(nc)
    return nc


_NC_CACHE = {}


def _get_nc():
    if "nc" not in _NC_CACHE:
        _NC_CACHE["nc"] = build_nc()
    return _NC_CACHE["nc"]


def make_in_maps(combined_embed, W_K, b_K, W_Q, b_Q, W_V, b_V, W_O, b_O):
    f32 = np.float32
    in_maps = []
    for c in range(8):
        b = c // 4
        g = c % 4
        sl = slice(g * 128, (g + 1) * 128)
        xt = np.ascontiguousarray(np.asarray(combined_embed[b], f32).T)
        wqkv = np.hstack(
            [
                np.asarray(W_Q, f32)[:, sl],
                np.asarray(W_K, f32)[:, sl],
                np.asarray(W_V, f32)[:, sl],
            ]
        )
        bqk = np.stack([np.asarray(b_Q, f32)[sl], np.asarray(b_K, f32)[sl]], 1)
        in_maps.append(
            {
                "xt": xt.astype(_BF16),
                "wqkv": np.ascontiguousarray(wqkv).astype(_BF16),
                "wo": np.ascontiguousarray(np.asarray(W_O, f32)[sl, :]).astype(
                    _BF16
                ),
                "bqk": np.ascontiguousarray(bqk),
                "bvp": np.asarray(b_V, f32)[sl].reshape(128, 1).copy(),
            }
        )
    return in_maps


def run_cores(in_maps, **kwargs):
    nc = _get_nc()
    return run_bass_kernel_spmd(nc, in_maps, core_ids=list(range(8)), **kwargs)


def kernel(
    combined_embed, W_K, b_K, W_Q, b_Q, W_V, b_V, W_O, b_O
):  # full inputs -> full output
    in_maps = make_in_maps(
        combined_embed, W_K, b_K, W_Q, b_Q, W_V, b_V, W_O, b_O
    )
    res = run_cores(in_maps)
    out = np.zeros((B, T, D), np.float32)
    for c in range(8):
        out[c // 4] += res.results[c]["out"]
    out += np.asarray(b_O, np.float32)[None, None, :]
    return out
